# revision 5
# baseline (speedup 1.0000x reference)
"""Trainium2 Bass kernel: causal self-attention with GQA + RoPE + sliding window.

Model (hardcoded from the problem spec):
  D_MODEL=2048, N_HEADS=16 (head_dim 128), N_KV_HEADS=4, T=2048, B=2,
  SLIDING_WINDOW=512, THETA=10000.

Sharding: 8 cores = batch(2) x kv-groups(4). Core (b, g) handles batch b and
query heads 4g..4g+3 with kv head g (Wqkv column-sharded). Output projection
is row-sharded (rows 512g..512g+512); the 4 partial products per batch are
summed on the host.

On-chip layout is feature-major ("transposed"): x is fed pre-transposed
(host-side) as xT [d_model, T], the QKV projection produces qkv^T
[d_out, tok], attention runs on S^T = K@Q^T tiles [k, q] so softmax
normalization uses a ones-vector matmul for the partition-dim sum, and the
PV product directly yields O^T [dv, q] which is the natural lhsT for the
output projection. All matmuls run in float32r (TF32-like, full PE rate at
moving-dim >= 256).
"""

import math

import numpy as np

try:
    import concourse.bass as bass
except ImportError:  # pragma: no cover - environment fallback
    import sys

    sys.path.insert(0, "/opt/trn_rl_repo")
    import concourse.bass as bass

import concourse.mybir as mybir
import concourse.tile as tile
from concourse import bacc
from concourse.bass_utils import run_bass_kernel_spmd

D_MODEL = 2048
N_HEADS = 16
N_KV_HEADS = 4
HEAD_DIM = 128
KV_DIM = N_KV_HEADS * HEAD_DIM  # 512
T = 2048
B = 2
SW = 512
THETA = 10000.0

P = 128
SB = 512                 # token super-block
N_SB = T // SB           # 4
KC = D_MODEL // P        # 16 contraction chunks
QH = 4                   # query heads per core
DOUT = QH * HEAD_DIM + 2 * HEAD_DIM  # 768 sharded qkv out dim
MQK = DOUT // P          # 6 dout chunks (0..3 Q, 4 K, 5 V)
SCALE = 1.0 / math.sqrt(HEAD_DIM)

F32 = mybir.dt.float32
F32R = mybir.dt.float32r

_CACHE = {}


def _build_program():
    nc = bacc.Bacc("TRN2", target_bir_lowering=False, debug=False, num_devices=8)

    xT = nc.dram_tensor("xT", [D_MODEL, T], F32R, kind="ExternalInput").ap()
    wqkv = nc.dram_tensor("wqkv", [D_MODEL, DOUT], F32R, kind="ExternalInput").ap()
    wout = nc.dram_tensor("wout", [QH * HEAD_DIM, D_MODEL], F32R, kind="ExternalInput").ap()
    cosT = nc.dram_tensor("cosT", [P, T], F32, kind="ExternalInput").ap()
    sinS = nc.dram_tensor("sinS", [P, T], F32, kind="ExternalInput").ap()
    m0 = nc.dram_tensor("m0", [P, P], F32R, kind="ExternalInput").ap()
    m4 = nc.dram_tensor("m4", [P, P], F32R, kind="ExternalInput").ap()
    ones = nc.dram_tensor("ones", [P, 1], F32R, kind="ExternalInput").ap()
    ident = nc.dram_tensor("ident", [P, P], F32, kind="ExternalInput").ap()
    y = nc.dram_tensor("y", [T, D_MODEL], F32, kind="ExternalOutput").ap()

    with tile.TileContext(nc) as tc:
        with (
            tc.tile_pool(name="const", bufs=1) as cpool,
            tc.tile_pool(name="work", bufs=2) as wpool,
            tc.tile_pool(name="psum", bufs=6, space="PSUM") as pspool,
        ):
            # --- resident tensors -------------------------------------------------
            wq_t = cpool.tile([P, KC, DOUT], F32R, tag="wqkv")
            nc.sync.dma_start(wq_t[:], wqkv.rearrange("(c p) m -> p c m", p=P))
            wo_t = cpool.tile([P, QH, D_MODEL], F32R, tag="wout")
            nc.sync.dma_start(wo_t[:], wout.rearrange("(c p) n -> p c n", p=P))
            cos_t = cpool.tile([P, T], F32, tag="cosT")
            nc.sync.dma_start(cos_t[:], cosT[:])
            sin_t = cpool.tile([P, T], F32, tag="sinS")
            nc.sync.dma_start(sin_t[:], sinS[:])
            m0_t = cpool.tile([P, P], F32R, tag="m0")
            nc.sync.dma_start(m0_t[:], m0[:])
            m4_t = cpool.tile([P, P], F32R, tag="m4")
            nc.sync.dma_start(m4_t[:], m4[:])
            ones_t = cpool.tile([P, 1], F32R, tag="ones")
            nc.sync.dma_start(ones_t[:], ones[:])
            id_t = cpool.tile([P, P], F32, tag="ident")
            nc.sync.dma_start(id_t[:], ident[:])

            k_res = cpool.tile([P, T], F32R, tag="k_res")   # K^T rope'd [d, tok]
            v_res = cpool.tile([P, T], F32R, tag="v_res")   # V as tok-chunks [tok, dv]

            for a in range(N_SB):
                tok = slice(a * SB, (a + 1) * SB)

                # --- QKV projection: qkv^T chunk psums [dout 128, tok 512] -------
                ps_q = [pspool.tile([P, SB], F32, tag="ps", name=f"qkv_{a}_{m}")
                        for m in range(MQK)]
                for k in range(KC):
                    xk = wpool.tile([P, SB], F32R, tag="xT", bufs=4)
                    nc.sync.dma_start(xk[:], xT[k * P:(k + 1) * P, tok])
                    for m in range(MQK):
                        nc.tensor.matmul(
                            ps_q[m][:],
                            wq_t[:, k, m * P:(m + 1) * P],
                            xk[:],
                            start=(k == 0),
                            stop=(k == KC - 1),
                        )

                # --- RoPE on Q heads (m=0..3) and K (m=4) ------------------------
                q_cur = wpool.tile([P, QH, SB], F32R, tag="q_cur", bufs=1)
                for m in range(5):
                    raw = wpool.tile([P, SB], F32, tag="rope_raw")
                    nc.scalar.copy(raw[:], ps_q[m][:])
                    rot = wpool.tile([P, SB], F32, tag="rope_rot")
                    nc.sync.dma_start(rot[0:64, :], raw[64:128, :])
                    nc.sync.dma_start(rot[64:128, :], raw[0:64, :])
                    t1 = wpool.tile([P, SB], F32, tag="rope_t1")
                    nc.vector.tensor_mul(out=t1[:], in0=raw[:], in1=cos_t[:, tok])
                    t2 = wpool.tile([P, SB], F32, tag="rope_t2")
                    nc.vector.tensor_mul(out=t2[:], in0=rot[:], in1=sin_t[:, tok])
                    dest = q_cur[:, m, :] if m < QH else k_res[:, tok]
                    nc.vector.tensor_add(out=dest, in0=t1[:], in1=t2[:])

                # --- V: transpose v^T [dv, tok] -> v [tok, dv] chunks ------------
                vraw = wpool.tile([P, SB], F32, tag="vraw")
                nc.scalar.copy(vraw[:], ps_q[5][:])
                for t in range(SB // P):
                    ptt = pspool.tile([P, P], F32, tag="ps2", bufs=2, name=f"tr_{a}_{t}")
                    nc.tensor.transpose(ptt[:], vraw[:, t * P:(t + 1) * P], id_t[:])
                    nc.scalar.copy(v_res[:, (4 * a + t) * P:(4 * a + t + 1) * P], ptt[:])

                # --- attention per head ------------------------------------------
                for h in range(QH):
                    acc = wpool.tile([P, SB], F32R, tag="acc", bufs=2)
                    ot_ps = pspool.tile([P, SB], F32, tag="ps", name=f"ot_{a}_{h}")
                    valid = [j for j in range(8) if 4 * a - 4 + j >= 0]
                    # j=4 spans all 512 q-columns; run it first so the PSUM
                    # bank-clearing start=True matmul covers the full bank.
                    jorder = [4] + [j for j in valid if j != 4]
                    covered = 0
                    for j in jorder:
                        ki = 4 * a - 4 + j
                        qlo = P * max(0, j - 4)
                        qhi = P * (min(3, j) + 1)
                        s_ps = pspool.tile([P, SB], F32, tag="ps", name=f"s_{a}_{h}_{j}")
                        nc.tensor.matmul(
                            s_ps[:, qlo:qhi],
                            k_res[:, ki * P:(ki + 1) * P],
                            q_cur[:, h, qlo:qhi],
                            start=True,
                            stop=True,
                        )
                        pT = wpool.tile([P, SB], F32R, tag="pT", bufs=3)
                        nc.scalar.activation(
                            pT[:, qlo:qhi], s_ps[:, qlo:qhi],
                            mybir.ActivationFunctionType.Exp, scale=SCALE,
                        )
                        if j <= 3:
                            seg = slice(j * P, (j + 1) * P)
                            nc.vector.tensor_mul(out=pT[:, seg], in0=pT[:, seg], in1=m4_t[:])
                        else:
                            seg = slice((j - 4) * P, (j - 3) * P)
                            nc.vector.tensor_mul(out=pT[:, seg], in0=pT[:, seg], in1=m0_t[:])
                        if qhi > covered:
                            nc.vector.tensor_copy(
                                out=acc[:, covered:qhi], in_=pT[:, covered:qhi])
                        hi = min(qhi, covered)
                        if qlo < hi:
                            nc.vector.tensor_add(
                                out=acc[:, qlo:hi], in0=acc[:, qlo:hi],
                                in1=pT[:, qlo:hi])
                        covered = max(covered, qhi)
                        nc.tensor.matmul(
                            ot_ps[:, qlo:qhi],
                            v_res[:, ki * P:(ki + 1) * P],
                            pT[:, qlo:qhi],
                            start=(j == jorder[0]),
                            stop=(j == jorder[-1]),
                        )
                    # softmax denominator: column sums via ones-vector matmul
                    sums = pspool.tile([1, SB], F32, tag="ps2", bufs=2, name=f"sum_{a}_{h}")
                    nc.tensor.matmul(sums[:], ones_t[:], acc[:], start=True, stop=True)
                    rrow = wpool.tile([1, SB], F32, tag="rrow")
                    nc.vector.reciprocal(rrow[:], sums[:])
                    rbc = wpool.tile([P, SB], F32, tag="rbc")
                    nc.gpsimd.partition_broadcast(rbc[:], rrow[:], channels=P)
                    ot_sb = wpool.tile([P, SB], F32R, tag=f"oT{h}", bufs=1)
                    nc.vector.tensor_mul(out=ot_sb[:], in0=ot_ps[:], in1=rbc[:])
                    if h == 0:
                        ot_all = [ot_sb]
                    else:
                        ot_all.append(ot_sb)

                # --- output projection: y[tok, :] partial ------------------------
                for t in range(SB // P):
                    for n in range(D_MODEL // SB):
                        py = pspool.tile([P, SB], F32, tag="ps", name=f"y_{a}_{t}_{n}")
                        for h in range(QH):
                            nc.tensor.matmul(
                                py[:],
                                ot_all[h][:, t * P:(t + 1) * P],
                                wo_t[:, h, n * SB:(n + 1) * SB],
                                start=(h == 0),
                                stop=(h == QH - 1),
                            )
                        yt = wpool.tile([P, SB], F32, tag="ytile", bufs=3)
                        nc.scalar.copy(yt[:], py[:])
                        nc.sync.dma_start(
                            y[a * SB + t * P: a * SB + (t + 1) * P, n * SB:(n + 1) * SB],
                            yt[:],
                        )

    nc.compile()
    return nc


def _host_tables():
    inv_freq = 1.0 / (THETA ** (np.arange(0, HEAD_DIM, 2, dtype=np.float32) / HEAD_DIM))
    pos = np.arange(T, dtype=np.float32)
    freqs = np.outer(pos, inv_freq)                     # [T, 64]
    emb = np.concatenate([freqs, freqs], axis=-1)       # [T, 128]
    cosT = np.ascontiguousarray(np.cos(emb).T.astype(np.float32))  # [128, T]
    sinT = np.sin(emb).T.astype(np.float32)
    sinS = sinT.copy()
    sinS[0:64] = -sinS[0:64]                            # rotate-half sign
    sinS = np.ascontiguousarray(sinS)
    kk = np.arange(P)[:, None]
    qq = np.arange(P)[None, :]
    m0 = (kk <= qq).astype(np.float32)                  # causal diag, [k, q] layout
    m4 = (kk > qq).astype(np.float32)                   # window edge
    ones = np.ones((P, 1), dtype=np.float32)
    ident = np.eye(P, dtype=np.float32)
    return cosT, sinS, m0, m4, ones, ident


def kernel(x, Wqkv, Wout):
    x = np.asarray(x, dtype=np.float32)
    Wqkv = np.asarray(Wqkv, dtype=np.float32)
    Wout = np.asarray(Wout, dtype=np.float32)

    if "nc" not in _CACHE:
        _CACHE["nc"] = _build_program()
    nc = _CACHE["nc"]

    cosT, sinS, m0, m4, ones, ident = _host_tables()
    xTs = [np.ascontiguousarray(x[b].T) for b in range(B)]

    in_maps = []
    for c in range(8):
        b, g = divmod(c, N_KV_HEADS)
        wq = Wqkv[:, g * QH * HEAD_DIM:(g + 1) * QH * HEAD_DIM]
        wk = Wqkv[:, D_MODEL + g * HEAD_DIM: D_MODEL + (g + 1) * HEAD_DIM]
        wv = Wqkv[:, D_MODEL + KV_DIM + g * HEAD_DIM: D_MODEL + KV_DIM + (g + 1) * HEAD_DIM]
        wqkv_sh = np.ascontiguousarray(np.concatenate([wq, wk, wv], axis=1))
        wout_sh = np.ascontiguousarray(Wout[g * QH * HEAD_DIM:(g + 1) * QH * HEAD_DIM])
        in_maps.append({
            "xT": xTs[b], "wqkv": wqkv_sh, "wout": wout_sh,
            "cosT": cosT, "sinS": sinS, "m0": m0, "m4": m4,
            "ones": ones, "ident": ident,
        })

    res = run_bass_kernel_spmd(nc, in_maps, core_ids=list(range(8)))

    y = np.zeros((B, T, D_MODEL), dtype=np.float32)
    for c in range(8):
        b = c // N_KV_HEADS
        y[b] += res.results[c]["y"]
    return y
